# revision 44
# baseline (speedup 1.0000x reference)
"""Trainium2 Bass kernel for BaseLUTLayer (probabilistic LUT node eval).

Math (per reference):
  x_eff = where(flip, 1 - x, x)                      # (B, IN)
  g[b,n,j] = x_eff[b, mapping[n,j]]                  # gather, (B, N, 6)
  out[b,n] = sum_k sigmoid(lut[n,k]) * prod_j (g_j if bit_j(k) else 1-g_j)

Device algorithm (centered-monomial basis):
  host:  t[b,i] = (x[b,i] - 0.5) * (1 - 2*flip[b,i])          (fp16, (IN,B))
         C[n,:] = centered-monomial transform of sigmoid(lut[n,:])
                  (out = sum_S C[n,S] * prod_{j in S} t_j, |t_j| <= 0.5)
  dev:   gather the 6 t-rows per node (dma_gather, one 768-descriptor
         gather per 128-node tile), then fold per tile:
           level 0:  U[m] = C[2m] + t0 * C[2m+1]     32 scalar-FMA rows
                     (fp32 per-partition scalars; TensorScalarPtr runs the
                     4x DVE perf mode, so DVE rows cost ~0.29 ns/col vs
                     0.92 on ACT and 1.4 on Pool -> rows split 12 DVE /
                     20 ACT)
           level j:  V = U_even + t_j * U_odd        mul+add fp16 tensor
                     rows (2x DVE perf mode); batch columns split into two
                     independent chains: [0:204) on DVE, [204:256) on Pool.

Engine balance (TimelineSim): DVE ~71us, Pool ~69us, ACT ~65us busy over
an ~83us critical path; all ops fp16 except the fp32 scalars and the final
level-5 add (fp32 output).

Sharding: nodes split 8 ways (1024 nodes/core); batch replicated.
Per-core output is (1024, 256) fp32, host concatenates + transposes.
"""

import numpy as np

B = 256
IN = 8192
NN = 8192
FAN = 6
NPAT = 64
NCORES = 8
PT = 128  # nodes per tile (partition dim)

# engine split tuning (see _build_nc)
N_ACT = 20        # level-0 FMA rows on ACT (of 32), per tile (int or list)
N_POOL = 0        # level-0 FMA rows on Pool, per tile (int or list)
W_DVE = 204       # batch columns of levels 1-5 on DVE (rest Pool), per tile
GGROUPS = (1,) * 8  # tiles per gather chunk (768 descs each; a gather
                    # instruction larger than the SWDGE descriptor ring
                    # overflows it and faults the device)
BUFS = 2          # work pool depth (pipeline tiles)
U_BUFS = 5        # U (level-0 output) pool depth
L1_CH = 2         # DVE-lane chunking of level 1

_CACHE = {}


def _per_tile(v, nt):
    return list(v) if isinstance(v, (list, tuple)) else [v] * nt


def _build_nc(nl, b, inp, n_act=N_ACT, n_pool=N_POOL, w_dve=W_DVE,
              ggroups=GGROUPS, bufs=BUFS, l1_ch=L1_CH, u_bufs=U_BUFS,
              warm=True, row_mode="block", idx_pool=False):
    """Build + compile the SPMD Bass program for one core's slice.

    Level-0 row assignment: rows are split DVE-first / ACT / Pool-last, and
    level 1 on the DVE lane is chunked in q so the first L1 chunk only
    depends on the early U rows.
    """
    import concourse.bacc as bacc
    import concourse.mybir as mybir
    from concourse.tile import TileContext
    from concourse._compat import get_trn_type

    dt = mybir.dt
    Alu = mybir.AluOpType
    Act = mybir.ActivationFunctionType

    nt = nl // PT              # tiles
    assert sum(ggroups) == nt
    n_act = _per_tile(n_act, nt)
    n_pool = _per_tile(n_pool, nt)
    w_dve = _per_tile(w_dve, nt)

    nc = bacc.Bacc(
        get_trn_type() or "TRN2",
        target_bir_lowering=False,
        debug=False,
        num_devices=NCORES,
    )
    tT = nc.dram_tensor("tT", [inp, b], dt.float16, kind="ExternalInput")
    # host-packed: Cpk[p, t*64+k] = C[t*128+p, k]
    Ctab = nc.dram_tensor("C", [128, nt * NPAT], dt.float32, kind="ExternalInput")
    n_idx = nl * FAN
    idx = nc.dram_tensor("idx", [128, n_idx // 16], dt.int16, kind="ExternalInput")
    outT = nc.dram_tensor("outT", [nl, b], dt.float32, kind="ExternalOutput")

    f16, f32 = dt.float16, dt.float32

    with TileContext(nc) as tc:
        with (
            tc.tile_pool(name="const", bufs=1) as cpool,
            tc.tile_pool(name="upool", bufs=u_bufs) as up,
            tc.tile_pool(name="work", bufs=bufs) as wk,
        ):
            idx_sb = cpool.tile([128, n_idx // 16], dt.int16)
            (nc.gpsimd if idx_pool else nc.sync).dma_start(idx_sb[:, :], idx[:, :])
            C_sb = cpool.tile([128, nt * NPAT], f32)
            nc.sync.dma_start(C_sb[:, :], Ctab[:, :])

            if warm:
                # warm the ACT function table before real work (1.3us load)
                warm_t = cpool.tile([128, 2], f16)
                nc.vector.memset(warm_t[:, :], 0.0)
                nc.scalar.activation(warm_t[:, :], warm_t[:, :], Act.Identity)

            # gathers: ggroups[G] tiles each; tile t -> (gather G, local tile tl)
            gt, t2g = [], {}
            t0i = 0
            iw0 = 0
            for G, tg in enumerate(ggroups):
                npg = PT * FAN * tg
                iw = npg // 16
                g = cpool.tile([128, tg * FAN, b], f16, tag=f"g{G}")
                nc.gpsimd.dma_gather(
                    g[:, :, :], tT[:, :], idx_sb[:, iw0:iw0 + iw],
                    npg, npg, b,
                )
                gt.append(g)
                for tl in range(tg):
                    t2g[t0i + tl] = (G, tl)
                t0i += tg
                iw0 += iw

            for t in range(nt):
                G, tl = t2g[t]
                a = lambda j: gt[G][:, tl * FAN + j, :]
                a3 = lambda j, sl: gt[G][:, tl * FAN + j:tl * FAN + j + 1, sl]
                Ct = C_sb[:, t * NPAT:(t + 1) * NPAT]
                nA, nP, wD = n_act[t], n_pool[t], w_dve[t]
                nD = 32 - nA - nP
                if row_mode == "odd_dve":
                    # DVE takes odd rows first (L1 mul inputs), ACT evens
                    row_eng = [""] * 32
                    order = list(range(1, 32, 2)) + list(range(0, 32, 2))
                    for i, m in enumerate(order):
                        row_eng[m] = "dve" if i < nD else ("act" if i < nD + nA else "pool")
                else:
                    row_eng = ["dve"] * nD + ["act"] * nA + ["pool"] * nP

                # --- level 0: U[m] = C[2m] + t0*C[2m+1], 32 scalar-FMA rows ---
                U = up.tile([128, 32, b], f16, tag="U")
                t0 = a(0)
                for m in range(32):
                    dst = U[:, m, :]
                    sc, bi = Ct[:, 2 * m + 1:2 * m + 2], Ct[:, 2 * m:2 * m + 1]
                    e = row_eng[m]
                    if e == "act":
                        nc.scalar.activation(dst, t0, Act.Identity, scale=sc, bias=bi)
                    elif e == "pool":
                        nc.gpsimd.tensor_scalar(
                            out=dst, in0=t0, scalar1=sc, scalar2=bi,
                            op0=Alu.mult, op1=Alu.add,
                        )
                    else:
                        nc.vector.tensor_scalar(
                            out=dst, in0=t0, scalar1=sc, scalar2=bi,
                            op0=Alu.mult, op1=Alu.add,
                        )

                # --- levels 1..5: V = U_even + t_j*U_odd, two independent
                # column lanes (uniform width keeps the chains decoupled) ---
                out_t = wk.tile([128, 1, b], f32, tag="out")
                lanes = []
                if wD > 0:
                    lanes.append((nc.vector, slice(0, wD), wD, "D"))
                if wD < b:
                    lanes.append((nc.gpsimd, slice(wD, b), b - wD, "P"))
                for eng, sl, w, nm in lanes:
                    V = U[:, :, sl]
                    for j in range(1, 6):
                        h = 32 >> j
                        P = wk.tile([128, h, w], f16, tag=f"P{j}{nm}")
                        if j < 5:
                            Vn = wk.tile([128, h, w], f16, tag=f"V{j}{nm}")
                        else:
                            Vn = out_t[:, :, sl]
                        ch = l1_ch if (j == 1 and nm == "D") else 1
                        hc = h // ch
                        for c in range(ch):
                            qs = slice(c * hc, (c + 1) * hc)
                            lo, hi = 2 * c * hc, 2 * (c + 1) * hc
                            tjb = a3(j, sl).broadcast_to([128, hc, w])
                            eng.tensor_mul(P[:, qs, :], V[:, lo + 1:hi:2, :], tjb)
                            eng.tensor_add(Vn[:, qs, :], P[:, qs, :], V[:, lo:hi:2, :])
                        if j < 5:
                            V = Vn

                nc.sync.dma_start(outT[t * PT:(t + 1) * PT, :], out_t[:, 0, :])

    nc.compile()
    return nc


def _prep_core_inputs(x, lut_table, mapping, flip_mask, nl, b, inp, n_cores=NCORES, ggroups=GGROUPS):
    """Host-side layout prep: t-table, centered-monomial tables, packed indices."""
    x = np.asarray(x)
    flip = np.asarray(flip_mask)
    # t[b,i] = (x-0.5)*(1-2f), transposed to (IN, B) fp16 for the gather
    tT = np.ascontiguousarray(
        ((x - 0.5) * (1.0 - 2.0 * flip)).T.astype(np.float16)
    )

    # centered-monomial transform of sigmoid(lut): out = sum_S C_S prod_{j in S} t_j
    lut64 = np.asarray(lut_table, dtype=np.float64)
    s = 1.0 / (1.0 + np.exp(-lut64))
    C = s.reshape(-1, 2, 2, 2, 2, 2, 2)  # axes [N, b5, b4, b3, b2, b1, b0]
    for j in range(6):
        ax = 1 + (5 - j)
        e = np.take(C, 0, axis=ax)
        o = np.take(C, 1, axis=ax)
        C = np.stack([0.5 * (e + o), o - e], axis=ax)
    C = C.reshape(-1, NPAT).astype(np.float32)

    nt = nl // PT
    in_maps = []
    for c in range(n_cores):
        sl = slice(c * nl, (c + 1) * nl)
        m_c = np.asarray(mapping[sl])  # (nl, 6) int32
        # per gather G (covering tiles t0..t0+tg-1):
        #   local index j = (tl*6+f)*128 + p -> m_c[(t0+tl)*128+p, f]
        by_tile = m_c.reshape(nt, PT, FAN).transpose(0, 2, 1)  # (nt, FAN, PT)
        wraps = []
        t0i = 0
        for tg in ggroups:
            og = by_tile[t0i:t0i + tg].reshape(-1)
            w = np.ascontiguousarray(og.astype(np.int16).reshape(-1, 16).T)
            wraps.append(np.tile(w, (8, 1)))  # (128, iw)
            t0i += tg
        idx_full = np.concatenate(wraps, axis=1)
        # pack C: Cpk[p, t*64+k] = C[t*128+p, k]
        Cpk = np.ascontiguousarray(
            C[sl].reshape(nt, PT, NPAT).transpose(1, 0, 2).reshape(PT, nt * NPAT)
        )
        in_maps.append({"tT": tT, "C": Cpk, "idx": idx_full})
    return in_maps


def _run(nc, in_maps, **kw):
    from concourse.bass_utils import run_bass_kernel_spmd

    last = None
    for attempt in range(3):
        try:
            return run_bass_kernel_spmd(nc, in_maps, list(range(NCORES)), **kw)
        except Exception as e:  # transient device errors happen on this fabric
            last = e
            if "UNRECOVERABLE" not in str(e) and "UNAVAILABLE" not in str(e):
                raise
    raise last


def kernel(x, lut_table, mapping, flip_mask):
    b, inp = x.shape
    nn = lut_table.shape[0]
    nl = nn // NCORES
    key = (nl, b, inp)
    if key not in _CACHE:
        _CACHE[key] = _build_nc(nl, b, inp)
    nc = _CACHE[key]
    in_maps = _prep_core_inputs(x, lut_table, mapping, flip_mask, nl, b, inp)
    res = _run(nc, in_maps)
    outT = np.concatenate([res.results[c]["outT"] for c in range(NCORES)], axis=0)
    return np.ascontiguousarray(outT.T, dtype=np.float32)


# revision 49
# speedup vs baseline: 1.0063x; 1.0063x over previous
"""Trainium2 Bass kernel for BaseLUTLayer (probabilistic LUT node eval).

Math (per reference):
  x_eff = where(flip, 1 - x, x)                      # (B, IN)
  g[b,n,j] = x_eff[b, mapping[n,j]]                  # gather, (B, N, 6)
  out[b,n] = sum_k sigmoid(lut[n,k]) * prod_j (g_j if bit_j(k) else 1-g_j)

Device algorithm (centered-monomial basis):
  host:  t[b,i] = (x[b,i] - 0.5) * (1 - 2*flip[b,i])          (fp16, (IN,B))
         C[n,:] = centered-monomial transform of sigmoid(lut[n,:])
                  (out = sum_S C[n,S] * prod_{j in S} t_j, |t_j| <= 0.5)
  dev:   gather the 6 t-rows per node (dma_gather, one 768-descriptor
         gather per 128-node tile), then fold per tile:
           level 0:  U[m] = C[2m] + t0 * C[2m+1]     32 scalar-FMA rows
                     (fp32 per-partition scalars; TensorScalarPtr runs the
                     4x DVE perf mode, so DVE rows cost ~0.29 ns/col vs
                     0.92 on ACT and 1.4 on Pool -> rows split 12 DVE /
                     20 ACT)
           level j:  V = U_even + t_j * U_odd        mul+add fp16 tensor
                     rows (2x DVE perf mode); batch columns split into two
                     independent chains: [0:204) on DVE, [204:256) on Pool.

Engine balance (TimelineSim): DVE ~71us, Pool ~69us, ACT ~65us busy over
an ~83us critical path; all ops fp16 except the fp32 scalars and the final
level-5 add (fp32 output).

Sharding: nodes split 8 ways (1024 nodes/core); batch replicated.
Per-core output is (1024, 256) fp32, host concatenates + transposes.
"""

import numpy as np

B = 256
IN = 8192
NN = 8192
FAN = 6
NPAT = 64
NCORES = 8
PT = 128  # nodes per tile (partition dim)

# engine split tuning (see _build_nc)
N_ACT = 20        # level-0 FMA rows on ACT (of 32), per tile (int or list)
N_POOL = 0        # level-0 FMA rows on Pool, per tile (int or list)
W_DVE = 204       # batch columns of levels 1-5 on DVE (rest Pool), per tile
GGROUPS = (1,) * 8  # tiles per gather chunk (768 descs each; a gather
                    # instruction larger than the SWDGE descriptor ring
                    # overflows it and faults the device)
BUFS = 2          # work pool depth (pipeline tiles)
U_BUFS = 5        # U (level-0 output) pool depth
L1_CH = 1         # DVE-lane chunking of level 1

_CACHE = {}


def _per_tile(v, nt):
    return list(v) if isinstance(v, (list, tuple)) else [v] * nt


def _build_nc(nl, b, inp, n_act=N_ACT, n_pool=N_POOL, w_dve=W_DVE,
              ggroups=GGROUPS, bufs=BUFS, l1_ch=L1_CH, u_bufs=U_BUFS,
              warm=True, row_mode="block", idx_pool=False, split_g0=False):
    """Build + compile the SPMD Bass program for one core's slice.

    Level-0 row assignment: rows are split DVE-first / ACT / Pool-last, and
    level 1 on the DVE lane is chunked in q so the first L1 chunk only
    depends on the early U rows.
    """
    import concourse.bacc as bacc
    import concourse.mybir as mybir
    from concourse.tile import TileContext
    from concourse._compat import get_trn_type

    dt = mybir.dt
    Alu = mybir.AluOpType
    Act = mybir.ActivationFunctionType

    nt = nl // PT              # tiles
    assert sum(ggroups) == nt
    n_act = _per_tile(n_act, nt)
    n_pool = _per_tile(n_pool, nt)
    w_dve = _per_tile(w_dve, nt)

    nc = bacc.Bacc(
        get_trn_type() or "TRN2",
        target_bir_lowering=False,
        debug=False,
        num_devices=NCORES,
    )
    tT = nc.dram_tensor("tT", [inp, b], dt.float16, kind="ExternalInput")
    # host-packed: Cpk[p, t*64+k] = C[t*128+p, k]
    Ctab = nc.dram_tensor("C", [128, nt * NPAT], dt.float32, kind="ExternalInput")
    n_idx = nl * FAN
    idx = nc.dram_tensor("idx", [128, n_idx // 16], dt.int16, kind="ExternalInput")
    outT = nc.dram_tensor("outT", [nl, b], dt.float32, kind="ExternalOutput")

    f16, f32 = dt.float16, dt.float32

    with TileContext(nc) as tc:
        with (
            tc.tile_pool(name="const", bufs=1) as cpool,
            tc.tile_pool(name="upool", bufs=u_bufs) as up,
            tc.tile_pool(name="work", bufs=bufs) as wk,
        ):
            idx_sb = cpool.tile([128, n_idx // 16], dt.int16)
            # split the idx load so gather 0 only waits on its own slice
            iw_first = (PT if split_g0 else PT * FAN * ggroups[0]) // 16
            idx_q = nc.gpsimd if idx_pool else nc.sync
            idx_q.dma_start(idx_sb[:, :iw_first], idx[:, :iw_first])
            idx_q.dma_start(idx_sb[:, iw_first:], idx[:, iw_first:])
            C_sb = cpool.tile([128, nt * NPAT], f32)
            nc.sync.dma_start(C_sb[:, :], Ctab[:, :])

            if warm:
                # warm the ACT function table before real work (1.3us load)
                warm_t = cpool.tile([128, 2], f16)
                nc.vector.memset(warm_t[:, :], 0.0)
                nc.scalar.activation(warm_t[:, :], warm_t[:, :], Act.Identity)

            # gathers: ggroups[G] tiles each; tile t -> (gather G, local tile tl)
            gt, t2g = [], {}
            t0i = 0
            iw0 = 0
            for G, tg in enumerate(ggroups):
                npg = PT * FAN * tg
                iw = npg // 16
                g = cpool.tile([128, tg * FAN, b], f16, tag=f"g{G}")
                if G == 0 and split_g0 and tg == 1:
                    # fanin-0 rows first (128 descs) so level 0 starts early;
                    # the wrapped idx layout keeps fanin 0 in cols [0:8)
                    nc.gpsimd.dma_gather(
                        g[:, 0:1, :], tT[:, :], idx_sb[:, 0:PT // 16],
                        PT, PT, b,
                    )
                    nc.gpsimd.dma_gather(
                        g[:, 1:, :], tT[:, :], idx_sb[:, PT // 16:iw],
                        npg - PT, npg - PT, b,
                    )
                else:
                    nc.gpsimd.dma_gather(
                        g[:, :, :], tT[:, :], idx_sb[:, iw0:iw0 + iw],
                        npg, npg, b,
                    )
                gt.append(g)
                for tl in range(tg):
                    t2g[t0i + tl] = (G, tl)
                t0i += tg
                iw0 += iw

            for t in range(nt):
                G, tl = t2g[t]
                a = lambda j: gt[G][:, tl * FAN + j, :]
                a3 = lambda j, sl: gt[G][:, tl * FAN + j:tl * FAN + j + 1, sl]
                Ct = C_sb[:, t * NPAT:(t + 1) * NPAT]
                nA, nP, wD = n_act[t], n_pool[t], w_dve[t]
                nD = 32 - nA - nP
                if row_mode == "odd_dve":
                    # DVE takes odd rows first (L1 mul inputs), ACT evens
                    row_eng = [""] * 32
                    order = list(range(1, 32, 2)) + list(range(0, 32, 2))
                    for i, m in enumerate(order):
                        row_eng[m] = "dve" if i < nD else ("act" if i < nD + nA else "pool")
                else:
                    row_eng = ["dve"] * nD + ["act"] * nA + ["pool"] * nP

                # --- level 0: U[m] = C[2m] + t0*C[2m+1], 32 scalar-FMA rows ---
                U = up.tile([128, 32, b], f16, tag="U")
                t0 = a(0)
                for m in range(32):
                    dst = U[:, m, :]
                    sc, bi = Ct[:, 2 * m + 1:2 * m + 2], Ct[:, 2 * m:2 * m + 1]
                    e = row_eng[m]
                    if e == "act":
                        nc.scalar.activation(dst, t0, Act.Identity, scale=sc, bias=bi)
                    elif e == "pool":
                        nc.gpsimd.tensor_scalar(
                            out=dst, in0=t0, scalar1=sc, scalar2=bi,
                            op0=Alu.mult, op1=Alu.add,
                        )
                    else:
                        nc.vector.tensor_scalar(
                            out=dst, in0=t0, scalar1=sc, scalar2=bi,
                            op0=Alu.mult, op1=Alu.add,
                        )

                # --- levels 1..5: V = U_even + t_j*U_odd, two independent
                # column lanes (uniform width keeps the chains decoupled) ---
                out_t = wk.tile([128, 1, b], f32, tag="out")
                lanes = []
                if wD > 0:
                    lanes.append((nc.vector, slice(0, wD), wD, "D"))
                if wD < b:
                    lanes.append((nc.gpsimd, slice(wD, b), b - wD, "P"))
                for eng, sl, w, nm in lanes:
                    V = U[:, :, sl]
                    for j in range(1, 6):
                        h = 32 >> j
                        P = wk.tile([128, h, w], f16, tag=f"P{j}{nm}")
                        if j < 5:
                            Vn = wk.tile([128, h, w], f16, tag=f"V{j}{nm}")
                        else:
                            Vn = out_t[:, :, sl]
                        ch = l1_ch if (j == 1 and nm == "D") else 1
                        hc = h // ch
                        for c in range(ch):
                            qs = slice(c * hc, (c + 1) * hc)
                            lo, hi = 2 * c * hc, 2 * (c + 1) * hc
                            tjb = a3(j, sl).broadcast_to([128, hc, w])
                            eng.tensor_mul(P[:, qs, :], V[:, lo + 1:hi:2, :], tjb)
                            eng.tensor_add(Vn[:, qs, :], P[:, qs, :], V[:, lo:hi:2, :])
                        if j < 5:
                            V = Vn

                nc.sync.dma_start(outT[t * PT:(t + 1) * PT, :], out_t[:, 0, :])

    nc.compile()
    return nc


def _prep_core_inputs(x, lut_table, mapping, flip_mask, nl, b, inp, n_cores=NCORES, ggroups=GGROUPS):
    """Host-side layout prep: t-table, centered-monomial tables, packed indices."""
    x = np.asarray(x)
    flip = np.asarray(flip_mask)
    # t[b,i] = (x-0.5)*(1-2f), transposed to (IN, B) fp16 for the gather
    tT = np.ascontiguousarray(
        ((x - 0.5) * (1.0 - 2.0 * flip)).T.astype(np.float16)
    )

    # centered-monomial transform of sigmoid(lut): out = sum_S C_S prod_{j in S} t_j
    lut64 = np.asarray(lut_table, dtype=np.float64)
    s = 1.0 / (1.0 + np.exp(-lut64))
    C = s.reshape(-1, 2, 2, 2, 2, 2, 2)  # axes [N, b5, b4, b3, b2, b1, b0]
    for j in range(6):
        ax = 1 + (5 - j)
        e = np.take(C, 0, axis=ax)
        o = np.take(C, 1, axis=ax)
        C = np.stack([0.5 * (e + o), o - e], axis=ax)
    C = C.reshape(-1, NPAT).astype(np.float32)

    nt = nl // PT
    in_maps = []
    for c in range(n_cores):
        sl = slice(c * nl, (c + 1) * nl)
        m_c = np.asarray(mapping[sl])  # (nl, 6) int32
        # per gather G (covering tiles t0..t0+tg-1):
        #   local index j = (tl*6+f)*128 + p -> m_c[(t0+tl)*128+p, f]
        by_tile = m_c.reshape(nt, PT, FAN).transpose(0, 2, 1)  # (nt, FAN, PT)
        wraps = []
        t0i = 0
        for tg in ggroups:
            og = by_tile[t0i:t0i + tg].reshape(-1)
            w = np.ascontiguousarray(og.astype(np.int16).reshape(-1, 16).T)
            wraps.append(np.tile(w, (8, 1)))  # (128, iw)
            t0i += tg
        idx_full = np.concatenate(wraps, axis=1)
        # pack C: Cpk[p, t*64+k] = C[t*128+p, k]
        Cpk = np.ascontiguousarray(
            C[sl].reshape(nt, PT, NPAT).transpose(1, 0, 2).reshape(PT, nt * NPAT)
        )
        in_maps.append({"tT": tT, "C": Cpk, "idx": idx_full})
    return in_maps


def _run(nc, in_maps, **kw):
    from concourse.bass_utils import run_bass_kernel_spmd

    last = None
    for attempt in range(3):
        try:
            return run_bass_kernel_spmd(nc, in_maps, list(range(NCORES)), **kw)
        except Exception as e:  # transient device errors happen on this fabric
            last = e
            if "UNRECOVERABLE" not in str(e) and "UNAVAILABLE" not in str(e):
                raise
    raise last


def kernel(x, lut_table, mapping, flip_mask):
    b, inp = x.shape
    nn = lut_table.shape[0]
    nl = nn // NCORES
    key = (nl, b, inp)
    if key not in _CACHE:
        _CACHE[key] = _build_nc(nl, b, inp)
    nc = _CACHE[key]
    in_maps = _prep_core_inputs(x, lut_table, mapping, flip_mask, nl, b, inp)
    res = _run(nc, in_maps)
    outT = np.concatenate([res.results[c]["outT"] for c in range(NCORES)], axis=0)
    return np.ascontiguousarray(outT.T, dtype=np.float32)
